# revision 5
# baseline (speedup 1.0000x reference)
"""Trainium2 Bass kernel for nn_Attention_41472204210940.

Reference computation (per batch b):
    q = x @ Wq; k, v = split(x @ Wkv); multi-head attention (H=8, DH=64);
    out = attn_out @ Wout + bout.

Sharding over 8 NeuronCores: core c handles batch b = c//2 and head group
g = c%2 (heads 4g..4g+4: inner-dim columns 256g..256g+256 of Wq/Wk/Wv
column-parallel, rows 256g..256g+256 of Wout row-parallel).  Each core
emits a partial [2048, 512] output; the host sums the two partials per
batch and adds bout.

Per-core program (bf16 matmul operands, fp32 PSUM accumulation):
  - QT/KT = W.T @ xT in [inner, N] layout; V natural [N, inner] plus a
    ones column per head so P @ V_aug also yields softmax denominators.
  - per (head-pair, query-block, key-chunk): ST[j, i] = K^T Q, then
    P = exp(SCALE * ST + mask_bias[j]).  Most key chunks run the exact
    exp on the Scalar (ACT) engine; a fixed subset per block runs on
    the DVE via a Schraudolph bit trick (i16 = rne(ST*(SCALE*128*log2e)
    + 16256 - C + mask_shift), reinterpreted bf16 == exp with ~1.8% RMS
    sawtooth, unbiased at C=7.3).  Softmax denominators absorb the
    common scale; errors average out across keys.  This matters because
    exp on ACT alone (~139us) matches the PE's total matmul streaming
    (~139us): offloading 1/4 makes the PE the sole pacer.
  - OT[d, i] += V_aug.T @ P accumulated over key chunks in PSUM; row DH
    holds the denominators.  Epilogue: denominator row is evacuated to
    SBUF (DVE), inverted on the otherwise-idle Pool engine with a bf16
    bit-trick seed + two Newton steps (no PSUM access on Pool, exact to
    2e-5), then DVE multiplies OT rows by the broadcast reciprocal AP
    (partition-stride-0) into AOT -- the odd head bounces through a
    bf16 tile + gpsimd SBUF DMA to reach partitions 64..127.  This
    replaces the 3.4us-per-head DVE reciprocal whose queue latency
    previously stalled the PE through psum-slot recycling.
  - out[t] = sum_pair AOT_pair[:, t].T @ Wout_pair, K=128 chains,
    evacuated alternately on ACT/DVE, psum alternating mm/ot pools.

Phases are sequential (projections, attention pass 0, pass 1, output
projection): with exp offloaded the kernel is PE-bound end to end, so
interleaving projection work into attention blocks cannot shrink the
span -- it only adds psum-slot contention (measured +70us).
"""

import numpy as np

B, N, D = 4, 2048, 512
H_TOTAL, DH = 8, 64
HEADS = 4            # heads per core
INNER = HEADS * DH   # per-core inner width (256)
N_CORES = 8
SCALE = DH ** -0.5


def build_program(n=N, d=D, heads=HEADS, dh=DH,
                  trick_jcs=(3, 7, 11, 15), trick_c=7.3,
                  nr_epilogue=True, p_bufs=10,
                  attn_prio=True, warmup_mms=12):
    """Build + compile the per-core Bass program (SPMD; all cores run the
    identical program on different data)."""
    import contextlib
    import concourse.bacc as bacc
    import concourse.mybir as mybir
    from concourse import tile

    f32 = mybir.dt.float32
    bf = mybir.dt.bfloat16
    i16 = mybir.dt.int16
    u8 = mybir.dt.uint8
    AF = mybir.ActivationFunctionType
    Alu = mybir.AluOpType

    inner = heads * dh
    KC = d // 128          # k-chunks of the projection contraction dim
    IC = inner // 128      # 128-row chunks of QT/KT == head pairs
    NJ = n // 128          # key chunks
    NI = n // 512          # query blocks
    VW = dh + 1            # V columns per head incl. the ones column

    LOG2E = 1.4426950408889634
    trick_a = SCALE * 128.0 * LOG2E          # logit -> bf16-bit scale
    s2_unmask = 16256.0 - trick_c            # 127<<7 minus sawtooth offset
    MASK_DROP = 42000.0                      # masked -> i16 ~ -25744 (tiny neg bf16)
    RECIP_K = 32496.0                        # bf16 reciprocal seed magic
    trick_set = frozenset(trick_jcs)

    assert dh == 64 and inner % 128 == 0 and n % 512 == 0 and d % 128 == 0

    nc = bacc.Bacc("TRN2", target_bir_lowering=False, debug=False)

    xt_d = nc.dram_tensor("xt", [d, n], bf, kind="ExternalInput")
    wq_d = nc.dram_tensor("wq", [d, inner], bf, kind="ExternalInput")
    wk_d = nc.dram_tensor("wk", [d, inner], bf, kind="ExternalInput")
    wv_d = nc.dram_tensor("wv", [d, inner], bf, kind="ExternalInput")
    wo_d = nc.dram_tensor("wo", [inner, d], bf, kind="ExternalInput")
    mask_d = nc.dram_tensor("mask", [n], u8, kind="ExternalInput")
    out_d = nc.dram_tensor("out", [n, d], f32, kind="ExternalOutput")

    with tile.TileContext(nc) as tc:
        with (
            nc.allow_low_precision(reason="bf16 matmul operand prep"),
            tc.tile_pool(name="const", bufs=1) as cpool,
            tc.tile_pool(name="pwork", bufs=p_bufs) as ppool,
            tc.tile_pool(name="small", bufs=2) as spool,
            tc.tile_pool(name="outsb", bufs=3) as opool,
            tc.tile_pool(name="mm", bufs=2, space="PSUM") as mmpool,
            tc.tile_pool(name="ot", bufs=2, space="PSUM") as otpool,
        ):
            # ---- input loads (bf16 from the host shard step) ----
            xTa = cpool.tile([128, KC * n], bf, name="xTa")
            wqa = cpool.tile([128, KC * inner], bf, name="wqa")
            wka = cpool.tile([128, KC * inner], bf, name="wka")
            wva = cpool.tile([128, KC * inner], bf, name="wva")
            wo = [cpool.tile([128, d], bf, name=f"wo{i}") for i in range(IC)]

            def xT(k):
                return xTa[:, n * k:n * (k + 1)]

            def wslice(wa, k):
                return wa[:, inner * k:inner * (k + 1)]

            masku8 = cpool.tile([128, NJ], u8, name="masku8")
            nc.sync.dma_start(
                out=masku8[:], in_=mask_d[:].rearrange("(c p) -> p c", p=128)
            )
            xt_r = xt_d[:].rearrange("(k p) c -> p k c", p=128)
            for t in range(NI):
                ts = slice(512 * t, 512 * (t + 1))
                nc.sync.dma_start(
                    out=xTa[:].rearrange("p (k c) -> p k c", c=n)[:, :, ts],
                    in_=xt_r[:, :, ts],
                )
                if t == 0:
                    for wa, wd in ((wqa, wq_d), (wka, wk_d)):
                        nc.scalar.dma_start(
                            out=wa[:].rearrange("p (k c) -> p k c", c=inner),
                            in_=wd[:].rearrange("(k p) c -> p k c", p=128),
                        )
                if t == min(1, NI - 1):
                    nc.sync.dma_start(
                        out=wva[:].rearrange("p (k c) -> p k c", c=inner),
                        in_=wv_d[:].rearrange("(k p) c -> p k c", p=128),
                    )
            for i in range(IC):
                nc.sync.dma_start(out=wo[i][:], in_=wo_d[128 * i:128 * (i + 1), :])

            # PE warmup: trip the HAM clock gate to 2.4GHz during DMA wait
            if warmup_mms:
                wup = cpool.tile([128, 512], bf, name="wup")
                nc.vector.memset(wup[:], 0.0)
                wps = mmpool.tile([128, 512], f32, tag="mm", name="wps")
                for i in range(warmup_mms):
                    nc.tensor.matmul(
                        wps[:], wup[:, 0:128], wup[:],
                        start=(i == 0), stop=(i == warmup_mms - 1),
                    )

            # ACT-exp bias: 0 if kept, -1e30 if masked
            maskb = cpool.tile([128, NJ], f32, name="maskb")
            nc.vector.tensor_scalar(
                maskb[:], masku8[:], -1.0, 1e30, Alu.add, Alu.mult
            )
            # DVE bit-trick exp bias: 16256-C if kept, shifted to tiny-neg if masked
            s2 = cpool.tile([128, NJ], f32, name="s2")
            nc.vector.tensor_scalar(
                s2[:], masku8[:], MASK_DROP, s2_unmask - MASK_DROP,
                Alu.mult, Alu.add
            )

            onesh_f = cpool.tile([128, heads], f32, name="onesh_f")
            nc.vector.memset(onesh_f[:], 1.0)

            QT = [cpool.tile([128, n], bf, name=f"QT{m}") for m in range(IC)]
            KT = [cpool.tile([128, n], bf, name=f"KT{m}") for m in range(IC)]
            V = [cpool.tile([128, heads * VW], bf, name=f"V{j}") for j in range(NJ)]
            AOT = [cpool.tile([128, n], bf, name=f"AOT{m}") for m in range(IC)]

            # ---- sequential projections (psum cycles the free ot slots) ----
            def qk_proj_one(m, chain):
                W, OUT = ((wqa, QT), (wka, KT))[chain % 2]
                t = chain // 2
                ts = slice(512 * t, 512 * (t + 1))
                ps = otpool.tile([128, 512], f32, tag="ot", name="psqk")
                for k in range(KC):
                    nc.tensor.matmul(
                        ps[:],
                        wslice(W, k)[:, 128 * m:128 * (m + 1)],
                        xT(k)[:, ts],
                        start=(k == 0),
                        stop=(k == KC - 1),
                    )
                nc.vector.tensor_copy(OUT[m][:, ts], ps[:])

            def v_proj(j):
                ps = otpool.tile([128, inner], f32, tag="ot", name="psv")
                for k in range(KC):
                    nc.tensor.matmul(
                        ps[:],
                        xT(k)[:, 128 * j:128 * (j + 1)],
                        wslice(wva, k),
                        start=(k == 0),
                        stop=(k == KC - 1),
                    )
                vv = V[j][:].rearrange("p (h e) -> p h e", e=VW)
                nc.vector.tensor_copy(
                    vv[:, :, 0:dh], ps[:].rearrange("p (h v) -> p h v", v=dh)
                )
                nc.gpsimd.tensor_copy(
                    vv[:, :, dh:VW],
                    onesh_f[:].rearrange("p (h o) -> p h o", o=1),
                )

            def final_proj(t):
                if t % 2 == 0:
                    ps = mmpool.tile([128, d], f32, tag="mm", name="psf")
                else:
                    ps = otpool.tile([128, d], f32, tag="ot", name="psf")
                for ic in range(IC):
                    nc.tensor.matmul(
                        ps[:],
                        AOT[ic][:, 128 * t:128 * (t + 1)],
                        wo[ic][:],
                        start=(ic == 0),
                        stop=(ic == IC - 1),
                    )
                ob = opool.tile([128, d], f32, tag="ob", name="ob")
                if t % 2 == 1:
                    nc.scalar.activation(ob[:], ps[:], AF.Copy)
                else:
                    nc.vector.tensor_copy(ob[:], ps[:])
                nc.sync.dma_start(out=out_d[128 * t:128 * (t + 1), :], in_=ob[:])

            for m in range(IC):
                for t in range(NI):
                    for chain in (0, 1):
                        qk_proj_one(m, 2 * t + chain)
                if m == 0:
                    for j in range(NJ):
                        v_proj(j)

            # ---- attention ----
            def pool_recip(dst, src):
                """dst[1,n] = 1/src[1,n] on the Pool engine: bf16 bit-trick
                seed + 2 Newton steps (max rel err ~2e-5)."""
                w = src.shape[1]
                dbf = spool.tile([1, w], bf, tag="dbf", name="dbf")
                nc.gpsimd.tensor_copy(dbf[:], src)
                r0i = spool.tile([1, w], i16, tag="r0i", name="r0i")
                nc.gpsimd.tensor_scalar(
                    r0i[:], dbf[:].bitcast(i16), -1.0, RECIP_K,
                    Alu.mult, Alu.add
                )
                r0 = r0i[:].bitcast(bf)
                t1 = spool.tile([1, w], f32, tag="t1", name="t1")
                nc.gpsimd.tensor_tensor(t1[:], src, r0, Alu.mult)
                nc.gpsimd.tensor_scalar(t1[:], t1[:], -1.0, 2.0, Alu.mult, Alu.add)
                r1 = spool.tile([1, w], f32, tag="r1", name="r1")
                nc.gpsimd.tensor_tensor(r1[:], t1[:], r0, Alu.mult)
                nc.gpsimd.tensor_tensor(t1[:], src, r1[:], Alu.mult)
                nc.gpsimd.tensor_scalar(t1[:], t1[:], -1.0, 2.0, Alu.mult, Alu.add)
                nc.gpsimd.tensor_tensor(dst, t1[:], r1[:], Alu.mult)

            def attn_block(ih, pr):
                isl = slice(512 * ih, 512 * (ih + 1))
                ot = otpool.tile([VW, 1024], f32, tag="ot", name="ot")
                for jc in range(NJ):
                    jsl = slice(128 * jc, 128 * (jc + 1))
                    st = mmpool.tile([128, 1024], f32, tag="mm", name="st")
                    for hh in range(2):
                        rsl = slice(64 * hh, 64 * (hh + 1))
                        nc.tensor.matmul(
                            st[:, 512 * hh:512 * (hh + 1)],
                            KT[pr][rsl, jsl],
                            QT[pr][rsl, isl],
                            start=True,
                            stop=True,
                        )
                    if jc in trick_set:
                        pi = ppool.tile([128, 1024], i16, tag="p", name="pi")
                        nc.vector.tensor_scalar(
                            pi[:], st[:], trick_a, s2[:, jc:jc + 1],
                            Alu.mult, Alu.add
                        )
                        p_ap = pi[:].bitcast(bf)
                    else:
                        p = ppool.tile([128, 1024], bf, tag="p", name="p")
                        nc.scalar.activation(
                            p[:], st[:], AF.Exp,
                            bias=maskb[:, jc:jc + 1], scale=SCALE,
                        )
                        p_ap = p[:]
                    for hh in range(2):
                        h = 2 * pr + hh
                        nc.tensor.matmul(
                            ot[:, 512 * hh:512 * (hh + 1)],
                            V[jc][:, VW * h:VW * (h + 1)],
                            p_ap[:, 512 * hh:512 * (hh + 1)],
                            start=(jc == 0),
                            stop=(jc == NJ - 1),
                        )
                # normalize: AOT rows = OT rows 0..dh-1 times 1/denom
                if nr_epilogue:
                    dsb = spool.tile([1, 1024], f32, tag="dsb", name="dsb")
                    nc.vector.tensor_copy(dsb[:], ot[dh:VW, :])
                    rc = spool.tile([1, 1024], f32, tag="rc2", name="rc2")
                    pool_recip(rc[:], dsb[:])
                    for hh in range(2):
                        csl = slice(512 * hh, 512 * (hh + 1))
                        rcb = spool.tile([dh, 512], f32, tag="rcb", name="rcb")
                        nc.gpsimd.partition_broadcast(rcb[:], rc[0:1, csl])
                        if hh == 0:
                            nc.vector.tensor_mul(
                                AOT[pr][0:dh, isl], ot[0:dh, csl], rcb[:]
                            )
                        else:
                            tb = spool.tile([dh, 512], bf, tag="tb", name="tb")
                            nc.vector.tensor_mul(tb[:], ot[0:dh, csl], rcb[:])
                            nc.gpsimd.dma_start(
                                out=AOT[pr][64:128, isl], in_=tb[:]
                            )
                else:
                    for hh in range(2):
                        csl = slice(512 * hh, 512 * (hh + 1))
                        rc = spool.tile([1, 512], f32, tag="rc", name="rc")
                        nc.vector.reciprocal(rc[:], ot[dh:VW, csl])
                        rcb = spool.tile([dh, 512], f32, tag="rcb", name="rcb")
                        nc.gpsimd.partition_broadcast(rcb[:], rc[:])
                        if hh == 0:
                            nc.vector.tensor_mul(
                                AOT[pr][0:dh, isl], ot[0:dh, csl], rcb[:]
                            )
                        else:
                            tb = spool.tile([dh, 512], bf, tag="tb", name="tb")
                            nc.vector.tensor_mul(tb[:], ot[0:dh, csl], rcb[:])
                            nc.gpsimd.dma_start(
                                out=AOT[pr][64:128, isl], in_=tb[:]
                            )

            prio_ctx = tc.high_priority if attn_prio else contextlib.nullcontext
            for pr in range(IC):
                for ih in range(NI):
                    with prio_ctx():
                        attn_block(ih, pr)

            # ---- output projection ----
            for t in range(4 * NI):
                final_proj(t)

    nc.compile()
    return nc


_PROGRAM = None


def _get_program():
    global _PROGRAM
    if _PROGRAM is None:
        _PROGRAM = build_program()
    return _PROGRAM


def make_in_maps(x, mask, Wq, Wkv, Wout):
    """Host-side shard: slice + lay out the full inputs for each core.
    Matmul operands ship as bf16 (the same round-to-nearest-even a device
    cast would apply before a bf16 matmul)."""
    import ml_dtypes

    bf16 = ml_dtypes.bfloat16
    in_maps = []
    for c in range(N_CORES):
        b, g = c // 2, c % 2
        cs = slice(INNER * g, INNER * (g + 1))
        vs = slice(D + INNER * g, D + INNER * (g + 1))
        in_maps.append({
            "xt": np.ascontiguousarray(x[b].T.astype(bf16)),
            "wq": np.ascontiguousarray(Wq[:, cs].astype(bf16)),
            "wk": np.ascontiguousarray(Wkv[:, cs].astype(bf16)),
            "wv": np.ascontiguousarray(Wkv[:, vs].astype(bf16)),
            "wo": np.ascontiguousarray(Wout[cs, :].astype(bf16)),
            "mask": np.ascontiguousarray(mask[b]).astype(np.uint8),
        })
    return in_maps


def combine_outputs(results, bout):
    """Host-side unshard: sum the two row-parallel partials per batch, add bias."""
    out = np.zeros((B, N, D), np.float32)
    for c in range(N_CORES):
        out[c // 2] += results[c]["out"]
    out += np.asarray(bout, np.float32)[None, None, :]
    return out


def kernel(**inputs):
    x = np.asarray(inputs["x"], np.float32)
    mask = np.asarray(inputs["mask"])
    Wq = np.asarray(inputs["Wq"], np.float32)
    Wkv = np.asarray(inputs["Wkv"], np.float32)
    Wout = np.asarray(inputs["Wout"], np.float32)
    bout = np.asarray(inputs["bout"], np.float32)

    from concourse.bass_utils import run_bass_kernel_spmd

    nc = _get_program()
    in_maps = make_in_maps(x, mask, Wq, Wkv, Wout)
    res = run_bass_kernel_spmd(nc, in_maps, list(range(N_CORES))).results
    return combine_outputs(res, bout)


if __name__ == "__main__":
    rng = np.random.default_rng(0)
    s = 1.0 / np.sqrt(D)
    demo = {
        "x": rng.standard_normal((B, N, D), np.float32),
        "mask": np.ones((B, N), bool),
        "Wq": rng.uniform(-s, s, (D, INNER * 2)).astype(np.float32),
        "Wkv": rng.uniform(-s, s, (D, INNER * 4)).astype(np.float32),
        "Wout": rng.uniform(-s, s, (INNER * 2, D)).astype(np.float32),
        "bout": rng.uniform(-s, s, D).astype(np.float32),
    }
    out = kernel(**demo)
    print("kernel output", out.shape, out.dtype, float(np.abs(out).max()))


# revision 7
# speedup vs baseline: 1.7288x; 1.7288x over previous
"""Trainium2 Bass kernel for nn_Attention_41472204210940.

Reference computation (per batch b):
    q = x @ Wq; k, v = split(x @ Wkv); multi-head attention (H=8, DH=64);
    out = attn_out @ Wout + bout.

Sharding over 8 NeuronCores: core c handles batch b = c//2 and head group
g = c%2 (heads 4g..4g+4: inner-dim columns 256g..256g+256 of Wq/Wk/Wv
column-parallel, rows 256g..256g+256 of Wout row-parallel).  Each core
emits a partial [2048, 512] output; the host sums the two partials per
batch and adds bout.

Per-core program (bf16 matmul operands, fp32 PSUM accumulation):
  - QT/KT = W.T @ xT in [inner, N] layout; V natural [N, inner] plus a
    ones column per head so P @ V_aug also yields softmax denominators.
  - per (head-pair, query-block, key-chunk): ST[j, i] = K^T Q, then
    P = exp(SCALE * ST + mask_bias[j]).  Most key chunks run the exact
    exp on the Scalar (ACT) engine; a fixed subset per block runs on
    the DVE via a Schraudolph bit trick (i16 = rne(ST*(SCALE*128*log2e)
    + 16256 - C + mask_shift), reinterpreted bf16 == exp with ~1.8% RMS
    sawtooth, unbiased at C=7.3).  Softmax denominators absorb the
    common scale; errors average out across keys.  This matters because
    exp on ACT alone (~139us) matches the PE's total matmul streaming
    (~139us): offloading 1/4 makes the PE the sole pacer.
  - OT[d, i] += V_aug.T @ P accumulated over key chunks in PSUM; row DH
    holds the denominators.  Epilogue: denominator row is evacuated to
    SBUF (DVE), inverted on the otherwise-idle Pool engine with a bf16
    bit-trick seed + two Newton steps (no PSUM access on Pool, exact to
    2e-5), then DVE multiplies OT rows by the broadcast reciprocal AP
    (partition-stride-0) into AOT -- the odd head bounces through a
    bf16 tile + gpsimd SBUF DMA to reach partitions 64..127.  This
    replaces the 3.4us-per-head DVE reciprocal whose queue latency
    previously stalled the PE through psum-slot recycling.
  - out[t] = sum_pair AOT_pair[:, t].T @ Wout_pair, K=128 chains,
    evacuated alternately on ACT/DVE, psum alternating mm/ot pools.

Phases are sequential (projections, attention pass 0, pass 1, output
projection): with exp offloaded the kernel is PE-bound end to end, so
interleaving projection work into attention blocks cannot shrink the
span -- it only adds psum-slot contention (measured +70us).
"""

import numpy as np

B, N, D = 4, 2048, 512
H_TOTAL, DH = 8, 64
HEADS = 4            # heads per core
INNER = HEADS * DH   # per-core inner width (256)
N_CORES = 8
SCALE = DH ** -0.5


def build_program(n=N, d=D, heads=HEADS, dh=DH,
                  trick_jcs=(3, 5, 8, 11, 13, 15), trick_c=7.3,
                  p_bufs=10,
                  attn_prio=True, warmup_mms=12):
    """Build + compile the per-core Bass program (SPMD; all cores run the
    identical program on different data)."""
    import contextlib
    import concourse.bacc as bacc
    import concourse.mybir as mybir
    from concourse import tile

    f32 = mybir.dt.float32
    bf = mybir.dt.bfloat16
    i16 = mybir.dt.int16
    u8 = mybir.dt.uint8
    AF = mybir.ActivationFunctionType
    Alu = mybir.AluOpType

    inner = heads * dh
    KC = d // 128          # k-chunks of the projection contraction dim
    IC = inner // 128      # 128-row chunks of QT/KT == head pairs
    NJ = n // 128          # key chunks
    NI = n // 512          # query blocks
    VW = dh + 1            # V columns per head incl. the ones column

    LOG2E = 1.4426950408889634
    trick_a = SCALE * 128.0 * LOG2E          # logit -> bf16-bit scale
    s2_unmask = 16256.0 - trick_c            # 127<<7 minus sawtooth offset
    MASK_DROP = 42000.0                      # masked -> i16 ~ -25744 (tiny neg bf16)
    RECIP_K = 32496.0                        # bf16 reciprocal seed magic
    trick_set = frozenset(trick_jcs)

    assert dh == 64 and inner % 128 == 0 and n % 512 == 0 and d % 128 == 0

    nc = bacc.Bacc("TRN2", target_bir_lowering=False, debug=False)

    xt_d = nc.dram_tensor("xt", [d, n], bf, kind="ExternalInput")
    wq_d = nc.dram_tensor("wq", [d, inner], bf, kind="ExternalInput")
    wk_d = nc.dram_tensor("wk", [d, inner], bf, kind="ExternalInput")
    wv_d = nc.dram_tensor("wv", [d, inner], bf, kind="ExternalInput")
    wo_d = nc.dram_tensor("wo", [inner, d], bf, kind="ExternalInput")
    mask_d = nc.dram_tensor("mask", [n], u8, kind="ExternalInput")
    out_d = nc.dram_tensor("out", [n, d], f32, kind="ExternalOutput")

    with tile.TileContext(nc) as tc:
        with (
            nc.allow_low_precision(reason="bf16 matmul operand prep"),
            tc.tile_pool(name="const", bufs=1) as cpool,
            tc.tile_pool(name="pwork", bufs=p_bufs) as ppool,
            tc.tile_pool(name="small", bufs=2) as spool,
            tc.tile_pool(name="outsb", bufs=3) as opool,
            tc.tile_pool(name="mm", bufs=2, space="PSUM") as mmpool,
            tc.tile_pool(name="ot", bufs=2, space="PSUM") as otpool,
        ):
            # ---- input loads (bf16 from the host shard step) ----
            xTa = cpool.tile([128, KC * n], bf, name="xTa")
            wqa = cpool.tile([128, KC * inner], bf, name="wqa")
            wka = cpool.tile([128, KC * inner], bf, name="wka")
            wva = cpool.tile([128, KC * inner], bf, name="wva")
            wo = [cpool.tile([128, d], bf, name=f"wo{i}") for i in range(IC)]

            def xT(k):
                return xTa[:, n * k:n * (k + 1)]

            def wslice(wa, k):
                return wa[:, inner * k:inner * (k + 1)]

            masku8 = cpool.tile([128, NJ], u8, name="masku8")
            nc.sync.dma_start(
                out=masku8[:], in_=mask_d[:].rearrange("(c p) -> p c", p=128)
            )
            xt_r = xt_d[:].rearrange("(k p) c -> p k c", p=128)
            for t in range(NI):
                ts = slice(512 * t, 512 * (t + 1))
                nc.sync.dma_start(
                    out=xTa[:].rearrange("p (k c) -> p k c", c=n)[:, :, ts],
                    in_=xt_r[:, :, ts],
                )
                if t == 0:
                    for wa, wd in ((wqa, wq_d), (wka, wk_d)):
                        nc.scalar.dma_start(
                            out=wa[:].rearrange("p (k c) -> p k c", c=inner),
                            in_=wd[:].rearrange("(k p) c -> p k c", p=128),
                        )
                if t == min(1, NI - 1):
                    nc.sync.dma_start(
                        out=wva[:].rearrange("p (k c) -> p k c", c=inner),
                        in_=wv_d[:].rearrange("(k p) c -> p k c", p=128),
                    )
            for i in range(IC):
                nc.sync.dma_start(out=wo[i][:], in_=wo_d[128 * i:128 * (i + 1), :])

            # PE warmup: trip the HAM clock gate to 2.4GHz during DMA wait
            if warmup_mms:
                wup = cpool.tile([128, 512], bf, name="wup")
                nc.vector.memset(wup[:], 0.0)
                wps = mmpool.tile([128, 512], f32, tag="mm", name="wps")
                for i in range(warmup_mms):
                    nc.tensor.matmul(
                        wps[:], wup[:, 0:128], wup[:],
                        start=(i == 0), stop=(i == warmup_mms - 1),
                    )

            # ACT-exp bias: 0 if kept, -1e30 if masked
            maskb = cpool.tile([128, NJ], f32, name="maskb")
            nc.vector.tensor_scalar(
                maskb[:], masku8[:], -1.0, 1e30, Alu.add, Alu.mult
            )
            # DVE bit-trick exp bias: 16256-C if kept, shifted to tiny-neg if masked
            s2 = cpool.tile([128, NJ], f32, name="s2")
            nc.vector.tensor_scalar(
                s2[:], masku8[:], MASK_DROP, s2_unmask - MASK_DROP,
                Alu.mult, Alu.add
            )

            onesh_f = cpool.tile([128, heads], f32, name="onesh_f")
            nc.vector.memset(onesh_f[:], 1.0)

            QT = [cpool.tile([128, n], bf, name=f"QT{m}") for m in range(IC)]
            KT = [cpool.tile([128, n], bf, name=f"KT{m}") for m in range(IC)]
            V = [cpool.tile([128, heads * VW], bf, name=f"V{j}") for j in range(NJ)]
            AOT = [cpool.tile([128, n], bf, name=f"AOT{m}") for m in range(IC)]

            # ---- sequential projections (psum cycles the free ot slots) ----
            def qk_proj_one(m, chain):
                W, OUT = ((wqa, QT), (wka, KT))[chain % 2]
                t = chain // 2
                ts = slice(512 * t, 512 * (t + 1))
                ps = otpool.tile([128, 512], f32, tag="ot", name="psqk")
                for k in range(KC):
                    nc.tensor.matmul(
                        ps[:],
                        wslice(W, k)[:, 128 * m:128 * (m + 1)],
                        xT(k)[:, ts],
                        start=(k == 0),
                        stop=(k == KC - 1),
                    )
                nc.vector.tensor_copy(OUT[m][:, ts], ps[:])

            def v_proj(j):
                ps = otpool.tile([128, inner], f32, tag="ot", name="psv")
                for k in range(KC):
                    nc.tensor.matmul(
                        ps[:],
                        xT(k)[:, 128 * j:128 * (j + 1)],
                        wslice(wva, k),
                        start=(k == 0),
                        stop=(k == KC - 1),
                    )
                vv = V[j][:].rearrange("p (h e) -> p h e", e=VW)
                nc.vector.tensor_copy(
                    vv[:, :, 0:dh], ps[:].rearrange("p (h v) -> p h v", v=dh)
                )
                nc.gpsimd.tensor_copy(
                    vv[:, :, dh:VW],
                    onesh_f[:].rearrange("p (h o) -> p h o", o=1),
                )

            def final_proj(t):
                if t % 2 == 0:
                    ps = mmpool.tile([128, d], f32, tag="mm", name="psf")
                else:
                    ps = otpool.tile([128, d], f32, tag="ot", name="psf")
                for ic in range(IC):
                    nc.tensor.matmul(
                        ps[:],
                        AOT[ic][:, 128 * t:128 * (t + 1)],
                        wo[ic][:],
                        start=(ic == 0),
                        stop=(ic == IC - 1),
                    )
                ob = opool.tile([128, d], f32, tag="ob", name="ob")
                if t % 2 == 1:
                    nc.scalar.activation(ob[:], ps[:], AF.Copy)
                else:
                    nc.vector.tensor_copy(ob[:], ps[:])
                nc.sync.dma_start(out=out_d[128 * t:128 * (t + 1), :], in_=ob[:])

            for m in range(IC):
                for t in range(NI):
                    for chain in (0, 1):
                        qk_proj_one(m, 2 * t + chain)
                if m == 0:
                    for j in range(NJ):
                        v_proj(j)

            # ---- attention, with a software-pipelined epilogue ----
            # Block g's normalization is spread over the next two blocks so
            # no engine queue ever gates the following block's critical ops:
            #   stage A (end of g):   DVE evacuates OT -> SBUF uot,
            #                         releasing the psum slot in ~1.2us.
            #   stage B (end of g+1): ACT Ln + Exp(scale=-1) computes
            #                         1/denominators (same act table as
            #                         exp: no table reload); Pool
            #                         partition-broadcasts them.
            #   stage C (inside g+2): DVE multiplies uot rows into AOT
            #                         (odd head via bf16 bounce + gpsimd
            #                         SBUF DMA to partitions 64..127).
            def attn_block(ih, pr, finish):
                isl = slice(512 * ih, 512 * (ih + 1))
                ot = otpool.tile([VW, 1024], f32, tag="ot", name="ot")
                for jc in range(NJ):
                    jsl = slice(128 * jc, 128 * (jc + 1))
                    st = mmpool.tile([128, 1024], f32, tag="mm", name="st")
                    for hh in range(2):
                        rsl = slice(64 * hh, 64 * (hh + 1))
                        nc.tensor.matmul(
                            st[:, 512 * hh:512 * (hh + 1)],
                            KT[pr][rsl, jsl],
                            QT[pr][rsl, isl],
                            start=True,
                            stop=True,
                        )
                    if jc in trick_set:
                        pi = ppool.tile([128, 1024], i16, tag="p", name="pi")
                        nc.vector.tensor_scalar(
                            pi[:], st[:], trick_a, s2[:, jc:jc + 1],
                            Alu.mult, Alu.add
                        )
                        p_ap = pi[:].bitcast(bf)
                    else:
                        p = ppool.tile([128, 1024], bf, tag="p", name="p")
                        nc.scalar.activation(
                            p[:], st[:], AF.Exp,
                            bias=maskb[:, jc:jc + 1], scale=SCALE,
                        )
                        p_ap = p[:]
                    for fn in finish.get(jc, ()):
                        fn()
                    for hh in range(2):
                        h = 2 * pr + hh
                        nc.tensor.matmul(
                            ot[:, 512 * hh:512 * (hh + 1)],
                            V[jc][:, VW * h:VW * (h + 1)],
                            p_ap[:, 512 * hh:512 * (hh + 1)],
                            start=(jc == 0),
                            stop=(jc == NJ - 1),
                        )
                # stage A: evacuate OT (incl. denominator row) to SBUF
                uot = spool.tile([VW, 1024], f32, tag="uot", bufs=3, name="uot")
                nc.vector.tensor_copy(uot[:], ot[:])
                state = {}

                def stage_b(uot=uot):
                    lnd = spool.tile([1, 1024], f32, tag="lnd", name="lnd")
                    nc.scalar.activation(lnd[:], uot[dh:VW, :], AF.Ln)
                    rc = spool.tile([1, 1024], f32, tag="rc2", bufs=3, name="rc2")
                    nc.scalar.activation(rc[:], lnd[:], AF.Exp, scale=-1.0)
                    for hh in range(2):
                        rcb = spool.tile(
                            [dh, 512], f32, tag="rcb", bufs=4, name="rcb"
                        )
                        nc.gpsimd.partition_broadcast(
                            rcb[:], rc[0:1, 512 * hh:512 * (hh + 1)]
                        )
                        state[hh] = rcb

                def stage_c0(uot=uot, pr=pr, isl=isl):
                    nc.vector.tensor_mul(
                        AOT[pr][0:dh, isl], uot[0:dh, 0:512], state[0][:]
                    )

                def stage_c1(uot=uot, pr=pr, isl=isl):
                    tb = spool.tile([dh, 512], bf, tag="tb", bufs=3, name="tb")
                    nc.vector.tensor_mul(tb[:], uot[0:dh, 512:1024], state[1][:])
                    nc.gpsimd.dma_start(out=AOT[pr][64:128, isl], in_=tb[:])

                return stage_b, (stage_c0, stage_c1)

            prio_ctx = tc.high_priority if attn_prio else contextlib.nullcontext
            blocks = [(ih, pr) for pr in range(IC) for ih in range(NI)]
            prev_b = None      # stage B of block g-1, emitted after block g
            prev_c = None      # stage C of block g-1, injected into block g+1
            inj_c = None       # stage C of block g-2, injected into block g
            for ih, pr in blocks:
                finish = {}
                if inj_c is not None:
                    finish[4] = (inj_c[0],)
                    finish[6] = (inj_c[1],)
                with prio_ctx():
                    b, c = attn_block(ih, pr, finish)
                    if prev_b is not None:
                        prev_b()
                inj_c = prev_c
                prev_b, prev_c = b, c

            # drain: stage B of the last block, stage C of the last two,
            # interleaved with the output projection so the first chunks
            # (which only need earlier query blocks' AOT) start at once.
            prev_b()
            for fn in inj_c:       # stage C of block 6 (queries 1024:1536)
                fn()
            for t in range(4 * NI - 4):
                final_proj(t)
            for fn in prev_c:      # stage C of block 7 (queries 1536:2048)
                fn()
            for t in range(4 * NI - 4, 4 * NI):
                final_proj(t)

    nc.compile()
    return nc


_PROGRAM = None


def _get_program():
    global _PROGRAM
    if _PROGRAM is None:
        _PROGRAM = build_program()
    return _PROGRAM


def make_in_maps(x, mask, Wq, Wkv, Wout):
    """Host-side shard: slice + lay out the full inputs for each core.
    Matmul operands ship as bf16 (the same round-to-nearest-even a device
    cast would apply before a bf16 matmul)."""
    import ml_dtypes

    bf16 = ml_dtypes.bfloat16
    in_maps = []
    for c in range(N_CORES):
        b, g = c // 2, c % 2
        cs = slice(INNER * g, INNER * (g + 1))
        vs = slice(D + INNER * g, D + INNER * (g + 1))
        in_maps.append({
            "xt": np.ascontiguousarray(x[b].T.astype(bf16)),
            "wq": np.ascontiguousarray(Wq[:, cs].astype(bf16)),
            "wk": np.ascontiguousarray(Wkv[:, cs].astype(bf16)),
            "wv": np.ascontiguousarray(Wkv[:, vs].astype(bf16)),
            "wo": np.ascontiguousarray(Wout[cs, :].astype(bf16)),
            "mask": np.ascontiguousarray(mask[b]).astype(np.uint8),
        })
    return in_maps


def combine_outputs(results, bout):
    """Host-side unshard: sum the two row-parallel partials per batch, add bias."""
    out = np.zeros((B, N, D), np.float32)
    for c in range(N_CORES):
        out[c // 2] += results[c]["out"]
    out += np.asarray(bout, np.float32)[None, None, :]
    return out


def kernel(**inputs):
    x = np.asarray(inputs["x"], np.float32)
    mask = np.asarray(inputs["mask"])
    Wq = np.asarray(inputs["Wq"], np.float32)
    Wkv = np.asarray(inputs["Wkv"], np.float32)
    Wout = np.asarray(inputs["Wout"], np.float32)
    bout = np.asarray(inputs["bout"], np.float32)

    from concourse.bass_utils import run_bass_kernel_spmd

    nc = _get_program()
    in_maps = make_in_maps(x, mask, Wq, Wkv, Wout)
    res = run_bass_kernel_spmd(nc, in_maps, list(range(N_CORES))).results
    return combine_outputs(res, bout)


if __name__ == "__main__":
    rng = np.random.default_rng(0)
    s = 1.0 / np.sqrt(D)
    demo = {
        "x": rng.standard_normal((B, N, D), np.float32),
        "mask": np.ones((B, N), bool),
        "Wq": rng.uniform(-s, s, (D, INNER * 2)).astype(np.float32),
        "Wkv": rng.uniform(-s, s, (D, INNER * 4)).astype(np.float32),
        "Wout": rng.uniform(-s, s, (INNER * 2, D)).astype(np.float32),
        "bout": rng.uniform(-s, s, D).astype(np.float32),
    }
    out = kernel(**demo)
    print("kernel output", out.shape, out.dtype, float(np.abs(out).max()))
